# revision 26
# baseline (speedup 1.0000x reference)
# DenseGATv2Conv Trainium2 kernel (v2).
#
# Math (per batch b):
#   xl = x @ W_l + b_l ; xr = x @ W_r + b_r            [N, H*C]
#   alpha[i,j,h] = sum_c att[h,c] * leaky_relu(xl[j,hc] + xr[i,hc], 0.2)
#   S = softmax_j(alpha masked by adj(+self loops))
#   out[i,hc] = sum_j S[i,j,h] * xr[j,hc] + bias
#
# Identities used on device:
#   leaky_relu(z) = 0.2*z + 0.8*relu(z)
#   alpha[i,j,h] = 0.2*sl[j,h] + 0.2*sr[i,h] + 0.8*sum_c att[h,c]*relu(xl[j,hc]+xr[i,hc])
# exp(0.2*sr[i,h]) cancels in the softmax; exp(0.2*sl[j,h]) (= esl) is folded
# multiplicatively into the aggregation operand.  The adjacency mask is applied
# ADDITIVELY pre-exp as 30*(adj-1) accumulated into the score PSUM by a small
# matmul, so masked entries underflow to 0 in the fp16 exp output.
#
# Per core: 256 dest rows = 2 ib x 4 supers x 32 rows.  Per super the 16
# dest-row pairs all accumulate into ONE [128, 1024] PSUM tile using 4
# stationary "variants" (att columns at local offset 8v) x 4 tile positions,
# so PSUM row r = 32q + 8v + 4d + h and dest-in-core = sup*32 + 8q + 2v + d
# comes out in natural order.  One exp per super writes fp16 scores which a
# DMA crossbar transpose scatters straight into the S^T aggregation layout.
#
# Sharding: 8 cores = (batch b in 0..1) x (4 blocks of 256 destination rows).

import numpy as np

B, N, F, H, C = 2, 1024, 128, 4, 16
HC = H * C
NCORES = 8
NI = 256          # destination rows per core
NSUP = 8          # supers of 16 pairs (32 dest rows) each
NF8 = 0           # pairs per super computed in fp8 (0/2/4); error ~5e-3/pair-pair

_CACHE = {}
LAST_RESULTS = None


def _build_program():
    import concourse.bass as bass
    import concourse.mybir as mybir
    import concourse.tile as tile
    from concourse import bacc

    f32 = mybir.dt.float32
    f16 = mybir.dt.float16
    f8 = mybir.dt.float8e4
    Alu = mybir.AluOpType
    Act = mybir.ActivationFunctionType

    nc = bacc.Bacc(
        "TRN2",
        target_bir_lowering=False,
        debug=False,
        enable_asserts=False,
        num_devices=NCORES,
    )

    # ---- DRAM I/O ----
    xbT16 = nc.dram_tensor("xbT16", [F, N], f16, kind="ExternalInput").ap()
    xisT16 = nc.dram_tensor("xisT16", [F, NI], f16, kind="ExternalInput").ap()
    adjx4 = nc.dram_tensor("adjx4", [128, NSUP * N], f16, kind="ExternalInput").ap()
    wl216 = nc.dram_tensor("wl216", [F, 128], f16, kind="ExternalInput").ap()
    wr16 = nc.dram_tensor("wr16", [F, HC], f16, kind="ExternalInput").ap()
    blp = nc.dram_tensor("blp", [128, 1], f32, kind="ExternalInput").ap()
    brp = nc.dram_tensor("brp", [HC, 1], f32, kind="ExternalInput").ap()
    attv = nc.dram_tensor("attv", [F, 128], f16, kind="ExternalInput").ap()
    attdr16 = nc.dram_tensor("attdr16", [F, 512], f16, kind="ExternalInput").ap()
    id16m = nc.dram_tensor("id16m", [128, 128], f16, kind="ExternalInput").ap()
    attbp = nc.dram_tensor("attbp", [HC, 16], f16, kind="ExternalInput").ap()
    brpb = nc.dram_tensor("brpb", [HC, 1], f32, kind="ExternalInput").ap()
    out = nc.dram_tensor("out", [NI, HC], f32, kind="ExternalOutput").ap()

    with tile.TileContext(nc) as tc:
        _body(tc, nc, mybir, f32, f16, f8, Alu, Act,
              xbT16, xisT16, adjx4, wl216, wr16, blp, brp, attv, attdr16, id16m, attbp,
              brpb, out)

    nc.compile()
    return nc


def _body(tc, nc, mybir, f32, f16, f8, Alu, Act,
          xbT16, xisT16, adjx4, wl216, wr16, blp, brp, attv, attdr16, id16m, attbp,
          brpb, out):
    from contextlib import ExitStack
    ctx = ExitStack()
    with ctx:
        consts = ctx.enter_context(tc.tile_pool(name="consts", bufs=1))
        work = ctx.enter_context(tc.tile_pool(name="work", bufs=1))
        rp_pool = ctx.enter_context(tc.tile_pool(name="rp", bufs=26))
        rp8_pool = ctx.enter_context(tc.tile_pool(name="rp8", bufs=5))
        sc_pool = ctx.enter_context(tc.tile_pool(name="sc", bufs=4))
        outp = ctx.enter_context(tc.tile_pool(name="outp", bufs=2))
        psg = ctx.enter_context(tc.tile_pool(name="psg", bufs=2, space="PSUM"))
        psb = ctx.enter_context(tc.tile_pool(name="psb", bufs=1, space="PSUM"))
        psa = ctx.enter_context(tc.tile_pool(name="psa", bufs=2, space="PSUM"))

        dma = nc.sync.dma_start
        dma2 = nc.scalar.dma_start      # Act HWDGE queue: output stores
        dmaT = nc.sync.dma_start_transpose

        # x^T arrives pre-transposed from the host, so startup is plain DMAs
        # on one queue, ordered by when the pipeline needs each tensor.
        xT = consts.tile([F, N], f16, tag="xT")       # [f, node]
        xisT = consts.tile([F, NI], f16, tag="xisT")  # [f, dest-slice node]
        wl2_t = consts.tile([F, 128], f16, tag="wl2")
        wr_t = consts.tile([F, HC], f16, tag="wr")
        blp2_t = consts.tile([128, 1], f32, tag="blp2")
        brpb_t = consts.tile([HC, 1], f32, tag="brpb")  # b_r + bias (xr_mod)
        brp_t = consts.tile([HC, 1], f32, tag="brp")
        attv_t = consts.tile([F, 128], f16, tag="attv")
        attdr_t = consts.tile([F, 512], f16, tag="attdr")
        att8_t = consts.tile([F, 512], f8, tag="att8")
        id16_t = consts.tile([128, 128], f16, tag="id16")
        attbp_t = consts.tile([HC, 16], f16, tag="attbp")
        adjx_t = consts.tile([128, NSUP * N], f16, tag="adjx")
        dma(xT[:, 0:512], xbT16[:, 0:512])
        dma(xT[:, 512:N], xbT16[:, 512:N])
        dma(wl2_t[:], wl216)
        dma(xisT[:], xisT16)
        dma(blp2_t[:], blp)
        dma(brp_t[:], brp)
        dma(attv_t[:], attv)
        dma(wr_t[:], wr16)
        dma(adjx_t[:], adjx4)
        dma(attbp_t[:], attbp)
        dma(brpb_t[:], brpb)
        dma(id16_t[:], id16m)
        if NF8:
            dma(attdr_t[:], attdr16)
            nc.vector.tensor_copy(att8_t[:], attdr_t[:])

        # ---------- projections ----------
        # xl2T: (x@W_l+b_l)^T stacked twice on partitions (for pair bias adds)
        xl2T = consts.tile([128, N], f16, tag="xl2T")
        xrT16 = consts.tile([HC, N], f16, tag="xrT16")   # (x@W_r+b_r)^T
        xrsT = consts.tile([HC, NI], f32, tag="xrsT")    # dest-row slice, f32
        pj = psg.tile([128, N], f32, tag="g", name="pj")
        for half in range(2):
            s = slice(half * 512, (half + 1) * 512)
            nc.tensor.matmul(pj[:, s], wl2_t[:], xT[:, s], start=True, stop=True)
        pj3 = psb.tile([HC, NI], f32, tag="b", name="pj3")
        nc.tensor.matmul(pj3[:], wr_t[:], xisT[:], start=True, stop=True)
        for half in range(2):
            s = slice(half * 512, (half + 1) * 512)
            nc.scalar.activation(xl2T[:, s], pj[:, s], Act.Identity,
                                 bias=blp2_t[:, 0:1], scale=1.0)
        nc.scalar.activation(xrsT[:], pj3[:], Act.Identity,
                             bias=brp_t[:, 0:1], scale=1.0)
        pj2 = psg.tile([HC, N], f32, tag="g", name="pj2")
        for half in range(2):
            s = slice(half * 512, (half + 1) * 512)
            nc.tensor.matmul(pj2[:, s], wr_t[:], xT[:, s], start=True, stop=True)
        nc.scalar.activation(xrT16[:], pj2[:], Act.Identity,
                             bias=brpb_t[:, 0:1], scale=1.0)

        # ---------- xrp: per-pair bias columns [xr[2p] ; xr[2p+1]] ----------
        xrp = consts.tile([128, 128], f32, tag="xrp")
        ev = xrsT[:].rearrange("p (a two) -> p a two", two=2)
        nc.vector.tensor_copy(xrp[0:HC, :], ev[:, :, 0])
        nc.vector.tensor_copy(xrp[HC:128, :], ev[:, :, 1])

        # ---------- xr_mod build: [j128, k, h, 0:16]=xr*esl, [..,16]=esl ----
        def build_xr_mod():
            # sl[h,j] = sum_hc att_blk[hc,h]*xl[hc,j]; esl = exp(0.2*sl)
            psl = psb.tile([16, N], f32, tag="b", name="psl")
            for half in range(2):
                s = slice(half * 512, (half + 1) * 512)
                nc.tensor.matmul(psl[:, s], attbp_t[:], xl2T[0:HC, s],
                                 start=True, stop=True)
            eslT = work.tile([16, N], f16, tag="eslT", name="eslT")
            nc.scalar.activation(eslT[:], psl[:], Act.Exp, scale=0.2)
            xr_nat = work.tile([128, 8 * HC], f16, tag="xrnat", name="xr_nat")
            esln = work.tile([128, 8 * 16], f16, tag="esln", name="esln")
            dmaT(xr_nat[:].rearrange("p (k c) -> p k c", k=8), xrT16[:])
            dmaT(esln[:].rearrange("p (k e) -> p k e", k=8), eslT[:])
            xmv = xr_mod[:].rearrange("p (k h e) -> p k h e", k=8, h=H)
            xnv = xr_nat[:].rearrange("p (k h c) -> p k h c", k=8, h=H)
            rep = esln[:].rearrange("p (k e) -> p k e", k=8)[:, :, 0:H]
            # broadcast esl over the 16 channels
            repb = esln[:].rearrange("p (k e one) -> p k e one", k=8, one=1)
            repb = repb[:, :, 0:H, :].broadcast_to([128, 8, H, C])
            nc.vector.tensor_tensor(xmv[:, :, :, 0:C], xnv, repb, Alu.mult)
            nc.vector.tensor_copy(xmv[:, :, :, C], rep)

        xr_mod = consts.tile([128, 8 * 68], f16, tag="xrmod")

        # ---------- main streaming loop ----------
        # st_t[ib]: S^T tiles, [j128, k*512 + s4*128 + r], r = PSUM row layout
        st_t = [consts.tile([128, 8 * 512], f16, tag=f"stt{ib}",
                            name=f"stt{ib}") for ib in range(2)]

        # ---------- aggregation ----------
        def aggregate(ib):
            out_f = outp.tile([128, HC], f32, tag="outf", name="outf")
            stv = st_t[ib][:].rearrange("p (k t h) -> p k t h", k=8, h=H)
            agg = psa.tile([128, 4 * 17], f32, tag="a", name="agg")
            for h in range(H):
                for k in range(8):
                    nc.tensor.matmul(agg[:, h * 17:(h + 1) * 17],
                                     stv[:, k, :, h],
                                     xr_mod[:, k * 68 + h * 17: k * 68 + (h + 1) * 17],
                                     start=(k == 0), stop=(k == 7))
            for h in range(H):
                rz = work.tile([128, 1], f32, tag="rz", name="rz")
                nc.vector.reciprocal(rz[:], agg[:, h * 17 + 16:h * 17 + 17])
                nc.vector.tensor_scalar(out_f[:, h * 16:(h + 1) * 16],
                                        agg[:, h * 17:h * 17 + 16], rz[:, 0:1],
                                        None, Alu.mult)
                dma2(out[ib * 128:(ib + 1) * 128, h * 16:(h + 1) * 16],
                     out_f[:, h * 16:(h + 1) * 16])

        for sup in range(NSUP):
            ib, s4 = sup // 4, sup % 4
            if sup == 1:
                build_xr_mod()
            if sup == 4:
                aggregate(0)
            gps = psg.tile([128, N], f32, tag="g", name=f"gps{sup}")
            # fp8 slots (b,u): each pair is one DoubleRow matmul with
            # ktile0 = fp8(att), ktile1 = fp8 residual of att, both k-tiles
            # streaming the same rp8 (stride-0 AP).  DoubleRow only supports
            # tile position (0,0), so fp8 slots live in PSUM rows 0..64.
            f8slots = [(0, 3), (1, 3), (0, 2), (1, 2)][:NF8]
            rp8s = []
            for (b8, u8) in f8slots:
                rp8 = rp8_pool.tile([128, N], f8, tag="rp8")
                p = sup * 16 + b8 * 4 + u8
                nc.scalar.activation(rp8[:], xl2T[:], Act.Relu,
                                     bias=xrp[:, p:p + 1], scale=1.0)
                rp8s.append(rp8[:].rearrange("p (one j) -> p one j", one=1))
            rps = {}
            for q in range(4):
                for v in range(4):
                    if (q, v) in f8slots:
                        continue
                    p = sup * 16 + q * 4 + v
                    rp = rp_pool.tile([128, N], f16, tag="rp")
                    nc.vector.tensor_scalar(rp[:], xl2T[:], xrp[:, p:p + 1],
                                            0.0, Alu.add, Alu.max)
                    rps[q, v] = rp
            for q in range(4):
                for v in range(4):
                    if (q, v) in f8slots:
                        continue
                    for half in range(2):
                        s = slice(half * 512, (half + 1) * 512)
                        nc.tensor.matmul(
                            gps[32 * q:32 * q + 32, s],
                            attv_t[:, 32 * v:32 * v + 32],
                            rps[q, v][:, s],
                            start=(v == 0), stop=(v == 3),
                            tile_position=(0, 32 * q),
                            skip_group_check=True,
                        )
            for si in range(NF8):
                for half in range(2):
                    s = slice(half * 512, (half + 1) * 512)
                    nc.tensor.matmul(
                        gps[0:64, s],
                        att8_t[:, 128 * si:128 * si + 128].rearrange(
                            "p (t m) -> p t m", t=2),
                        rp8s[si][:, :, s].broadcast_to([128, 2, 512]),
                        start=False, stop=(si == NF8 - 1),
                        perf_mode=mybir.MatmulPerfMode.DoubleRow,
                        tile_position=(0, 0),
                        skip_group_check=True,
                    )
            scomp = sc_pool.tile([128, N], f16, tag="scomp")
            scm = sc_pool.tile([128, N], f16, tag="scm")
            dstv = st_t[ib][:].rearrange("p (k s r) -> p k s r",
                                         k=8, s=4)
            for half in range(2):
                s = slice(half * 512, (half + 1) * 512)
                nc.scalar.activation(scomp[:, s], gps[:, s], Act.Exp)
                # adjacency mask (0/1, head-expanded) applied on vector engine
                nc.vector.tensor_tensor(
                    scm[:, s], scomp[:, s],
                    adjx_t[:, sup * N + half * 512: sup * N + half * 512 + 512],
                    Alu.mult)
                if sup == NSUP - 1:
                    # tail: PE transpose (short latency) instead of DMA xbar
                    for k in range(half * 4, half * 4 + 4):
                        pt = psa.tile([128, 128], f16, tag="a", name="pt")
                        nc.tensor.transpose(pt[:], scm[:, k * 128:(k + 1) * 128],
                                            id16_t[:])
                        nc.vector.tensor_copy(dstv[:, k, s4, :], pt[:])
                else:
                    dmaT(dstv[:, half * 4:(half + 1) * 4, s4, :], scm[:, s])

        aggregate(1)


def _get_program():
    if "nc" not in _CACHE:
        _CACHE["nc"] = _build_program()
    return _CACHE["nc"]


def kernel(x, adj, W_l, b_l, W_r, b_r, att, bias):
    global LAST_RESULTS
    from concourse.bass_utils import run_bass_kernel_spmd

    x = np.ascontiguousarray(np.asarray(x, dtype=np.float32))
    adj = np.ascontiguousarray(np.asarray(adj, dtype=np.float32))
    W_l = np.asarray(W_l, dtype=np.float32)
    b_l = np.asarray(b_l, dtype=np.float32)
    W_r = np.asarray(W_r, dtype=np.float32)
    b_r = np.asarray(b_r, dtype=np.float32)
    att = np.asarray(att, dtype=np.float32)
    bias = np.asarray(bias, dtype=np.float32)

    # host-side constant prep
    attv = np.zeros((F, 128), np.float32)
    for v in range(4):
        for d in range(2):
            for h in range(H):
                col = 32 * v + 8 * v + 4 * d + h
                attv[d * HC + h * C:d * HC + (h + 1) * C, col] = 0.8 * att[h]
    attv = attv.astype(np.float16)
    import ml_dtypes
    attdr = np.zeros((F, 4, 2, 64), np.float32)
    for si, (b8, u8) in enumerate([(0, 3), (1, 3), (0, 2), (1, 2)]):
        for d in range(2):
            for h in range(H):
                m = 32 * b8 + 8 * u8 + 4 * d + h
                a = 0.8 * att[h]
                amain = a.astype(ml_dtypes.float8_e4m3).astype(np.float32)
                ares = (a - amain).astype(ml_dtypes.float8_e4m3).astype(np.float32)
                attdr[d * HC + h * C:d * HC + (h + 1) * C, si, 0, m] = amain
                attdr[d * HC + h * C:d * HC + (h + 1) * C, si, 1, m] = ares
    attdr16 = attdr.reshape(F, 512).astype(np.float16)
    id16 = np.eye(128, dtype=np.float16)
    # PSUM row r = 32b+8u+4d+h  <->  dest-in-super ld = 8b+2u+d
    rowld = np.zeros(128, np.int64)
    for b8 in range(4):
        for u8 in range(4):
            for d in range(2):
                for h in range(H):
                    rowld[32 * b8 + 8 * u8 + 4 * d + h] = 8 * b8 + 2 * u8 + d
    attbp = np.zeros((HC, 16), np.float32)
    for h in range(H):
        attbp[h * C:(h + 1) * C, h] = att[h]
    attbp = attbp.astype(np.float16)
    blp = np.concatenate([b_l, b_l]).reshape(128, 1).astype(np.float32)
    brp = b_r.reshape(HC, 1).astype(np.float32).copy()
    brpb = (b_r + bias).reshape(HC, 1).astype(np.float32).copy()
    wl216 = np.concatenate([W_l, W_l], axis=1).astype(np.float16)
    wr16 = W_r.astype(np.float16).copy()

    in_maps = []
    for core in range(NCORES):
        b, blk = core // 4, core % 4
        i0 = blk * NI
        adjsl = adj[b, i0:i0 + NI, :].copy()
        adjsl[np.arange(NI), i0 + np.arange(NI)] = 1.0   # self loops
        # adjx4[r, sup*N+j] = adj[sup*32 + rowld[r], j]  (head-expanded 0/1)
        a3 = adjsl.reshape(NSUP, 32, N)[:, rowld, :]
        adjx = np.ascontiguousarray(a3.transpose(1, 0, 2)).reshape(128, NSUP * N)
        adjx = adjx.astype(np.float16)
        in_maps.append({
            "xbT16": np.ascontiguousarray(x[b].T).astype(np.float16),
            "xisT16": np.ascontiguousarray(x[b, i0:i0 + NI].T).astype(np.float16),
            "adjx4": adjx, "id16m": id16,
            "wl216": wl216, "wr16": wr16, "blp": blp, "brp": brp,
            "attv": attv, "attdr16": attdr16, "attbp": attbp,
            "brpb": brpb,
        })

    nc = _get_program()
    res = run_bass_kernel_spmd(nc, in_maps, core_ids=list(range(NCORES)))
    LAST_RESULTS = res
    outp = np.zeros((B, N, HC), np.float32)
    for core in range(NCORES):
        b, blk = core // 4, core % 4
        outp[b, blk * NI:(blk + 1) * NI, :] = res.results[core]["out"]
    return outp


# revision 27
# speedup vs baseline: 1.0151x; 1.0151x over previous
# DenseGATv2Conv Trainium2 kernel (v2).
#
# Math (per batch b):
#   xl = x @ W_l + b_l ; xr = x @ W_r + b_r            [N, H*C]
#   alpha[i,j,h] = sum_c att[h,c] * leaky_relu(xl[j,hc] + xr[i,hc], 0.2)
#   S = softmax_j(alpha masked by adj(+self loops))
#   out[i,hc] = sum_j S[i,j,h] * xr[j,hc] + bias
#
# Identities used on device:
#   leaky_relu(z) = 0.2*z + 0.8*relu(z)
#   alpha[i,j,h] = 0.2*sl[j,h] + 0.2*sr[i,h] + 0.8*sum_c att[h,c]*relu(xl[j,hc]+xr[i,hc])
# exp(0.2*sr[i,h]) cancels in the softmax; exp(0.2*sl[j,h]) (= esl) is folded
# multiplicatively into the aggregation operand.  The adjacency mask is applied
# ADDITIVELY pre-exp as 30*(adj-1) accumulated into the score PSUM by a small
# matmul, so masked entries underflow to 0 in the fp16 exp output.
#
# Per core: 256 dest rows = 2 ib x 4 supers x 32 rows.  Per super the 16
# dest-row pairs all accumulate into ONE [128, 1024] PSUM tile using 4
# stationary "variants" (att columns at local offset 8v) x 4 tile positions,
# so PSUM row r = 32q + 8v + 4d + h and dest-in-core = sup*32 + 8q + 2v + d
# comes out in natural order.  One exp per super writes fp16 scores which a
# DMA crossbar transpose scatters straight into the S^T aggregation layout.
#
# Sharding: 8 cores = (batch b in 0..1) x (4 blocks of 256 destination rows).

import numpy as np

B, N, F, H, C = 2, 1024, 128, 4, 16
HC = H * C
NCORES = 8
NI = 256          # destination rows per core
NSUP = 8          # supers of 16 pairs (32 dest rows) each
NF8 = 0           # pairs per super computed in fp8 (0/2/4); error ~5e-3/pair-pair

_CACHE = {}
LAST_RESULTS = None


def _build_program():
    import concourse.bass as bass
    import concourse.mybir as mybir
    import concourse.tile as tile
    from concourse import bacc

    f32 = mybir.dt.float32
    f16 = mybir.dt.float16
    f8 = mybir.dt.float8e4
    Alu = mybir.AluOpType
    Act = mybir.ActivationFunctionType

    nc = bacc.Bacc(
        "TRN2",
        target_bir_lowering=False,
        debug=False,
        enable_asserts=False,
        num_devices=NCORES,
    )

    # ---- DRAM I/O ----
    xbT16 = nc.dram_tensor("xbT16", [F, N], f16, kind="ExternalInput").ap()
    xisT16 = nc.dram_tensor("xisT16", [F, NI], f16, kind="ExternalInput").ap()
    adjx4 = nc.dram_tensor("adjx4", [128, NSUP * N], f16, kind="ExternalInput").ap()
    wl216 = nc.dram_tensor("wl216", [F, 128], f16, kind="ExternalInput").ap()
    wr16 = nc.dram_tensor("wr16", [F, HC], f16, kind="ExternalInput").ap()
    blp = nc.dram_tensor("blp", [128, 1], f32, kind="ExternalInput").ap()
    brp = nc.dram_tensor("brp", [HC, 1], f32, kind="ExternalInput").ap()
    attv = nc.dram_tensor("attv", [F, 128], f16, kind="ExternalInput").ap()
    attdr16 = nc.dram_tensor("attdr16", [F, 512], f16, kind="ExternalInput").ap()
    id16m = nc.dram_tensor("id16m", [128, 128], f16, kind="ExternalInput").ap()
    attbp = nc.dram_tensor("attbp", [HC, 16], f16, kind="ExternalInput").ap()
    brpb = nc.dram_tensor("brpb", [HC, 1], f32, kind="ExternalInput").ap()
    out = nc.dram_tensor("out", [NI, HC], f32, kind="ExternalOutput").ap()

    with tile.TileContext(nc) as tc:
        _body(tc, nc, mybir, f32, f16, f8, Alu, Act,
              xbT16, xisT16, adjx4, wl216, wr16, blp, brp, attv, attdr16, id16m, attbp,
              brpb, out)

    nc.compile()
    return nc


def _body(tc, nc, mybir, f32, f16, f8, Alu, Act,
          xbT16, xisT16, adjx4, wl216, wr16, blp, brp, attv, attdr16, id16m, attbp,
          brpb, out):
    from contextlib import ExitStack
    ctx = ExitStack()
    with ctx:
        consts = ctx.enter_context(tc.tile_pool(name="consts", bufs=1))
        work = ctx.enter_context(tc.tile_pool(name="work", bufs=1))
        rp_pool = ctx.enter_context(tc.tile_pool(name="rp", bufs=26))
        rp8_pool = ctx.enter_context(tc.tile_pool(name="rp8", bufs=5))
        sc_pool = ctx.enter_context(tc.tile_pool(name="sc", bufs=4))
        outp = ctx.enter_context(tc.tile_pool(name="outp", bufs=2))
        psg = ctx.enter_context(tc.tile_pool(name="psg", bufs=2, space="PSUM"))
        psb = ctx.enter_context(tc.tile_pool(name="psb", bufs=1, space="PSUM"))
        psa = ctx.enter_context(tc.tile_pool(name="psa", bufs=2, space="PSUM"))

        dma = nc.sync.dma_start
        dma2 = nc.scalar.dma_start      # Act HWDGE queue: output stores
        dmaT = nc.sync.dma_start_transpose

        # x^T arrives pre-transposed from the host, so startup is plain DMAs
        # on one queue, ordered by when the pipeline needs each tensor.
        xT = consts.tile([F, N], f16, tag="xT")       # [f, node]
        xisT = consts.tile([F, NI], f16, tag="xisT")  # [f, dest-slice node]
        wl2_t = consts.tile([F, 128], f16, tag="wl2")
        wr_t = consts.tile([F, HC], f16, tag="wr")
        blp2_t = consts.tile([128, 1], f32, tag="blp2")
        brpb_t = consts.tile([HC, 1], f32, tag="brpb")  # b_r + bias (xr_mod)
        brp_t = consts.tile([HC, 1], f32, tag="brp")
        attv_t = consts.tile([F, 128], f16, tag="attv")
        attdr_t = consts.tile([F, 512], f16, tag="attdr")
        att8_t = consts.tile([F, 512], f8, tag="att8")
        id16_t = consts.tile([128, 128], f16, tag="id16")
        attbp_t = consts.tile([HC, 16], f16, tag="attbp")
        adjx_t = consts.tile([128, NSUP * N], f16, tag="adjx")
        dma(xT[:, 0:512], xbT16[:, 0:512])
        dma(wl2_t[:], wl216)
        dma(xT[:, 512:N], xbT16[:, 512:N])
        dma(xisT[:], xisT16)
        dma(blp2_t[:], blp)
        dma(brp_t[:], brp)
        dma(attv_t[:], attv)
        dma(wr_t[:], wr16)
        dma(adjx_t[:], adjx4)
        dma(attbp_t[:], attbp)
        dma(brpb_t[:], brpb)
        dma(id16_t[:], id16m)
        if NF8:
            dma(attdr_t[:], attdr16)
            nc.vector.tensor_copy(att8_t[:], attdr_t[:])

        # ---------- projections ----------
        # xl2T: (x@W_l+b_l)^T stacked twice on partitions (for pair bias adds)
        xl2T = consts.tile([128, N], f16, tag="xl2T")
        xrT16 = consts.tile([HC, N], f16, tag="xrT16")   # (x@W_r+b_r)^T
        xrsT = consts.tile([HC, NI], f32, tag="xrsT")    # dest-row slice, f32
        pj = psg.tile([128, N], f32, tag="g", name="pj")
        for half in range(2):
            s = slice(half * 512, (half + 1) * 512)
            nc.tensor.matmul(pj[:, s], wl2_t[:], xT[:, s], start=True, stop=True)
        pj3 = psb.tile([HC, NI], f32, tag="b", name="pj3")
        nc.tensor.matmul(pj3[:], wr_t[:], xisT[:], start=True, stop=True)
        for half in range(2):
            s = slice(half * 512, (half + 1) * 512)
            nc.scalar.activation(xl2T[:, s], pj[:, s], Act.Identity,
                                 bias=blp2_t[:, 0:1], scale=1.0)
        nc.scalar.activation(xrsT[:], pj3[:], Act.Identity,
                             bias=brp_t[:, 0:1], scale=1.0)
        pj2 = psg.tile([HC, N], f32, tag="g", name="pj2")
        for half in range(2):
            s = slice(half * 512, (half + 1) * 512)
            nc.tensor.matmul(pj2[:, s], wr_t[:], xT[:, s], start=True, stop=True)
        nc.scalar.activation(xrT16[:], pj2[:], Act.Identity,
                             bias=brpb_t[:, 0:1], scale=1.0)

        # ---------- xrp: per-pair bias columns [xr[2p] ; xr[2p+1]] ----------
        xrp = consts.tile([128, 128], f32, tag="xrp")
        ev = xrsT[:].rearrange("p (a two) -> p a two", two=2)
        nc.vector.tensor_copy(xrp[0:HC, :], ev[:, :, 0])
        nc.vector.tensor_copy(xrp[HC:128, :], ev[:, :, 1])

        # ---------- xr_mod build: [j128, k, h, 0:16]=xr*esl, [..,16]=esl ----
        def build_xr_mod():
            # sl[h,j] = sum_hc att_blk[hc,h]*xl[hc,j]; esl = exp(0.2*sl)
            psl = psb.tile([16, N], f32, tag="b", name="psl")
            for half in range(2):
                s = slice(half * 512, (half + 1) * 512)
                nc.tensor.matmul(psl[:, s], attbp_t[:], xl2T[0:HC, s],
                                 start=True, stop=True)
            eslT = work.tile([16, N], f16, tag="eslT", name="eslT")
            nc.scalar.activation(eslT[:], psl[:], Act.Exp, scale=0.2)
            xr_nat = work.tile([128, 8 * HC], f16, tag="xrnat", name="xr_nat")
            esln = work.tile([128, 8 * 16], f16, tag="esln", name="esln")
            dmaT(xr_nat[:].rearrange("p (k c) -> p k c", k=8), xrT16[:])
            dmaT(esln[:].rearrange("p (k e) -> p k e", k=8), eslT[:])
            xmv = xr_mod[:].rearrange("p (k h e) -> p k h e", k=8, h=H)
            xnv = xr_nat[:].rearrange("p (k h c) -> p k h c", k=8, h=H)
            rep = esln[:].rearrange("p (k e) -> p k e", k=8)[:, :, 0:H]
            # broadcast esl over the 16 channels
            repb = esln[:].rearrange("p (k e one) -> p k e one", k=8, one=1)
            repb = repb[:, :, 0:H, :].broadcast_to([128, 8, H, C])
            nc.vector.tensor_tensor(xmv[:, :, :, 0:C], xnv, repb, Alu.mult)
            nc.vector.tensor_copy(xmv[:, :, :, C], rep)

        xr_mod = consts.tile([128, 8 * 68], f16, tag="xrmod")

        # ---------- main streaming loop ----------
        # st_t[ib]: S^T tiles, [j128, k*512 + s4*128 + r], r = PSUM row layout
        st_t = [consts.tile([128, 8 * 512], f16, tag=f"stt{ib}",
                            name=f"stt{ib}") for ib in range(2)]

        # ---------- aggregation ----------
        def aggregate(ib):
            out_f = outp.tile([128, HC], f32, tag="outf", name="outf")
            stv = st_t[ib][:].rearrange("p (k t h) -> p k t h", k=8, h=H)
            agg = psa.tile([128, 4 * 17], f32, tag="a", name="agg")
            for h in range(H):
                for k in range(8):
                    nc.tensor.matmul(agg[:, h * 17:(h + 1) * 17],
                                     stv[:, k, :, h],
                                     xr_mod[:, k * 68 + h * 17: k * 68 + (h + 1) * 17],
                                     start=(k == 0), stop=(k == 7))
            for h in range(H):
                rz = work.tile([128, 1], f32, tag="rz", name="rz")
                nc.vector.reciprocal(rz[:], agg[:, h * 17 + 16:h * 17 + 17])
                nc.vector.tensor_scalar(out_f[:, h * 16:(h + 1) * 16],
                                        agg[:, h * 17:h * 17 + 16], rz[:, 0:1],
                                        None, Alu.mult)
            dma2(out[ib * 128:(ib + 1) * 128, :], out_f[:])

        for sup in range(NSUP):
            ib, s4 = sup // 4, sup % 4
            if sup == 1:
                build_xr_mod()
            if sup == 4:
                aggregate(0)
            gps = psg.tile([128, N], f32, tag="g", name=f"gps{sup}")
            # fp8 slots (b,u): each pair is one DoubleRow matmul with
            # ktile0 = fp8(att), ktile1 = fp8 residual of att, both k-tiles
            # streaming the same rp8 (stride-0 AP).  DoubleRow only supports
            # tile position (0,0), so fp8 slots live in PSUM rows 0..64.
            f8slots = [(0, 3), (1, 3), (0, 2), (1, 2)][:NF8]
            rp8s = []
            for (b8, u8) in f8slots:
                rp8 = rp8_pool.tile([128, N], f8, tag="rp8")
                p = sup * 16 + b8 * 4 + u8
                nc.scalar.activation(rp8[:], xl2T[:], Act.Relu,
                                     bias=xrp[:, p:p + 1], scale=1.0)
                rp8s.append(rp8[:].rearrange("p (one j) -> p one j", one=1))
            rps = {}
            for q in range(4):
                for v in range(4):
                    if (q, v) in f8slots:
                        continue
                    p = sup * 16 + q * 4 + v
                    rp = rp_pool.tile([128, N], f16, tag="rp")
                    nc.vector.tensor_scalar(rp[:], xl2T[:], xrp[:, p:p + 1],
                                            0.0, Alu.add, Alu.max)
                    rps[q, v] = rp
            for q in range(4):
                for v in range(4):
                    if (q, v) in f8slots:
                        continue
                    for half in range(2):
                        s = slice(half * 512, (half + 1) * 512)
                        nc.tensor.matmul(
                            gps[32 * q:32 * q + 32, s],
                            attv_t[:, 32 * v:32 * v + 32],
                            rps[q, v][:, s],
                            start=(v == 0), stop=(v == 3),
                            tile_position=(0, 32 * q),
                            skip_group_check=True,
                        )
            for si in range(NF8):
                for half in range(2):
                    s = slice(half * 512, (half + 1) * 512)
                    nc.tensor.matmul(
                        gps[0:64, s],
                        att8_t[:, 128 * si:128 * si + 128].rearrange(
                            "p (t m) -> p t m", t=2),
                        rp8s[si][:, :, s].broadcast_to([128, 2, 512]),
                        start=False, stop=(si == NF8 - 1),
                        perf_mode=mybir.MatmulPerfMode.DoubleRow,
                        tile_position=(0, 0),
                        skip_group_check=True,
                    )
            scomp = sc_pool.tile([128, N], f16, tag="scomp")
            scm = sc_pool.tile([128, N], f16, tag="scm")
            dstv = st_t[ib][:].rearrange("p (k s r) -> p k s r",
                                         k=8, s=4)
            for half in range(2):
                s = slice(half * 512, (half + 1) * 512)
                nc.scalar.activation(scomp[:, s], gps[:, s], Act.Exp)
                # adjacency mask (0/1, head-expanded) applied on vector engine
                nc.vector.tensor_tensor(
                    scm[:, s], scomp[:, s],
                    adjx_t[:, sup * N + half * 512: sup * N + half * 512 + 512],
                    Alu.mult)
                if sup == NSUP - 1:
                    # tail: PE transpose (short latency) instead of DMA xbar
                    for k in range(half * 4, half * 4 + 4):
                        pt = psa.tile([128, 128], f16, tag="a", name="pt")
                        nc.tensor.transpose(pt[:], scm[:, k * 128:(k + 1) * 128],
                                            id16_t[:])
                        nc.vector.tensor_copy(dstv[:, k, s4, :], pt[:])
                else:
                    dmaT(dstv[:, half * 4:(half + 1) * 4, s4, :], scm[:, s])

        aggregate(1)


def _get_program():
    if "nc" not in _CACHE:
        _CACHE["nc"] = _build_program()
    return _CACHE["nc"]


def kernel(x, adj, W_l, b_l, W_r, b_r, att, bias):
    global LAST_RESULTS
    from concourse.bass_utils import run_bass_kernel_spmd

    x = np.ascontiguousarray(np.asarray(x, dtype=np.float32))
    adj = np.ascontiguousarray(np.asarray(adj, dtype=np.float32))
    W_l = np.asarray(W_l, dtype=np.float32)
    b_l = np.asarray(b_l, dtype=np.float32)
    W_r = np.asarray(W_r, dtype=np.float32)
    b_r = np.asarray(b_r, dtype=np.float32)
    att = np.asarray(att, dtype=np.float32)
    bias = np.asarray(bias, dtype=np.float32)

    # host-side constant prep
    attv = np.zeros((F, 128), np.float32)
    for v in range(4):
        for d in range(2):
            for h in range(H):
                col = 32 * v + 8 * v + 4 * d + h
                attv[d * HC + h * C:d * HC + (h + 1) * C, col] = 0.8 * att[h]
    attv = attv.astype(np.float16)
    import ml_dtypes
    attdr = np.zeros((F, 4, 2, 64), np.float32)
    for si, (b8, u8) in enumerate([(0, 3), (1, 3), (0, 2), (1, 2)]):
        for d in range(2):
            for h in range(H):
                m = 32 * b8 + 8 * u8 + 4 * d + h
                a = 0.8 * att[h]
                amain = a.astype(ml_dtypes.float8_e4m3).astype(np.float32)
                ares = (a - amain).astype(ml_dtypes.float8_e4m3).astype(np.float32)
                attdr[d * HC + h * C:d * HC + (h + 1) * C, si, 0, m] = amain
                attdr[d * HC + h * C:d * HC + (h + 1) * C, si, 1, m] = ares
    attdr16 = attdr.reshape(F, 512).astype(np.float16)
    id16 = np.eye(128, dtype=np.float16)
    # PSUM row r = 32b+8u+4d+h  <->  dest-in-super ld = 8b+2u+d
    rowld = np.zeros(128, np.int64)
    for b8 in range(4):
        for u8 in range(4):
            for d in range(2):
                for h in range(H):
                    rowld[32 * b8 + 8 * u8 + 4 * d + h] = 8 * b8 + 2 * u8 + d
    attbp = np.zeros((HC, 16), np.float32)
    for h in range(H):
        attbp[h * C:(h + 1) * C, h] = att[h]
    attbp = attbp.astype(np.float16)
    blp = np.concatenate([b_l, b_l]).reshape(128, 1).astype(np.float32)
    brp = b_r.reshape(HC, 1).astype(np.float32).copy()
    brpb = (b_r + bias).reshape(HC, 1).astype(np.float32).copy()
    wl216 = np.concatenate([W_l, W_l], axis=1).astype(np.float16)
    wr16 = W_r.astype(np.float16).copy()

    in_maps = []
    for core in range(NCORES):
        b, blk = core // 4, core % 4
        i0 = blk * NI
        adjsl = adj[b, i0:i0 + NI, :].copy()
        adjsl[np.arange(NI), i0 + np.arange(NI)] = 1.0   # self loops
        # adjx4[r, sup*N+j] = adj[sup*32 + rowld[r], j]  (head-expanded 0/1)
        a3 = adjsl.reshape(NSUP, 32, N)[:, rowld, :]
        adjx = np.ascontiguousarray(a3.transpose(1, 0, 2)).reshape(128, NSUP * N)
        adjx = adjx.astype(np.float16)
        in_maps.append({
            "xbT16": np.ascontiguousarray(x[b].T).astype(np.float16),
            "xisT16": np.ascontiguousarray(x[b, i0:i0 + NI].T).astype(np.float16),
            "adjx4": adjx, "id16m": id16,
            "wl216": wl216, "wr16": wr16, "blp": blp, "brp": brp,
            "attv": attv, "attdr16": attdr16, "attbp": attbp,
            "brpb": brpb,
        })

    nc = _get_program()
    res = run_bass_kernel_spmd(nc, in_maps, core_ids=list(range(NCORES)))
    LAST_RESULTS = res
    outp = np.zeros((B, N, HC), np.float32)
    for core in range(NCORES):
        b, blk = core // 4, core % 4
        outp[b, blk * NI:(blk + 1) * NI, :] = res.results[core]["out"]
    return outp
